# revision 1
# baseline (speedup 1.0000x reference)
"""Trainium2 Bass kernel for a 4-layer IndRNN (B=32, T=2048, I=256, H=512).

v2: custom DVE uOp programs (ANT_LSCAN2 / ANT_QREC2) compute the whole
recurrence h = relu(dloc - q) in two 1-elem/cycle passes over an interleaved
PAIR of batches, replacing the baseline's 2 stock scans + stt + relu
(~2.5x less DVE time, which is the bottleneck engine).

Math: per layer, with PSUM holding -xp (weights negated on host):
    l_t = w*l_{t-1} + (-xp_t)          (LSCAN2; l = -dloc)
    v_t = w*P_{t-1}; P_t = max(l_t, v_t); h_t = max(v_t - l_t, 0)  (QREC2)
which equals h_t = relu(xp_t + w*relu-recurrence), the IndRNN layer.

Pairing: two batches (2bp, 2bp+1) share each instruction (same per-channel
w), interleaved element-wise in SBUF (a0,b0,a1,b1,...). PE matmuls write
per-batch fp32 PSUM tiles; ACT copies de-/interleave into fp16 pair buffers;
next-layer matmuls read stride-2 rhs slices of the interleaved h pairs.

Sharding: data-parallel over batch, 4 batches (= 2 pairs) per core.
"""

import numpy as np

from concourse import dve_ops
from concourse.dve_spec import Spec, Src0, C0, relu as sp_relu
from concourse.dve_uop import (
    AluInp,
    AluOp,
    DelayInp,
    DveOpSpec,
    ENABLE,
    InpSel,
    OutPath,
    OutSel,
    Trigger,
    UopConfig,
    UopDpConfig,
)

# input lanes (lane k feeds PREV_DELAY_{k-1} at slice 0)
_LANE_X, _LANE_W, _LANE_Z = 0, 1, 2  # delay-chain ids for x, w, zero
_V_LANE = 3  # op2: captured v = w*P_{t-1}


def _base_uop():
    u = UopConfig()
    u.enable_input(InpSel.SRC_0, _LANE_X + 1)
    u.enable_input(InpSel.CONST_0, _LANE_W + 1)
    u.enable_input(InpSel.ZERO, _LANE_Z + 1)
    return u


def _seed_uop(next_idx: int) -> UopConfig:
    """repeat=2: two pipeline slots prime slice2/slice4 a-flops to 0."""
    u = _base_uop()
    u.require_inp0 = 0
    u.repeat_count = 2
    u.trigger = (Trigger.COUNT, Trigger.NONE, Trigger.NONE)
    u.next_uop = (next_idx, 0, 0)
    for k in range(4):
        u.datapath_config[k].pass_through_delay(_LANE_Z)
    for k in (2, 4):
        b = u.datapath_config[k]
        b.op = AluOp.BYPASS
        b.alu_src0 = AluInp.PREV_DELAY_0 + _LANE_Z
        b.alu_src1 = b.alu_src0
        b.alu_out_a_enable = ENABLE
    return u


def _finish_bypass(u: UopConfig, from_blk: int):
    """Blocks from_blk..7 pass the result along the out-flop chain; write
    block 7's ALU out."""
    for k in range(from_blk, 8):
        u.datapath_config[k].pass_through_alu()
    u.enable_output(OutSel.ALU_OUT, OutPath.WR0_LO)


def _lscan2_steady(my_mul: int, other_flop_wr: int, next_idx: int) -> UopConfig:
    """One stream's steady uop for LSCAN2. my_mul = slice of w*l_prev
    (feedback read); my_mul+1 = ADD slice writing the a_flop."""
    u = _base_uop()
    u.require_inp0 = 1
    u.repeat_count = 1
    u.trigger = (Trigger.SRC_TENSOR_DONE, Trigger.COUNT, Trigger.NONE)
    u.next_uop = (0, next_idx, 0)
    dp = u.datapath_config
    # carry x/w up to the mul/add slices
    for k in range(my_mul):
        dp[k].pass_through_delay(_LANE_X, _LANE_W)
    dp[my_mul].enable_alu(AluOp.MULTIPLY, AluInp.PREV_DELAY_0 + _LANE_W,
                          AluInp.NEXT_ALU_OUT_A)
    dp[my_mul].pass_through_delay(_LANE_X)
    add = dp[my_mul + 1].enable_alu(AluOp.ADD, AluInp.PREV_ALU_OUT,
                                    AluInp.PREV_DELAY_0 + _LANE_X)
    add.alu_out_a_enable = ENABLE
    _finish_bypass(u, my_mul + 2)
    return u


def _qrec2_steady(my_mul: int, next_idx: int) -> UopConfig:
    """One stream's steady uop for QREC2. my_mul: v = w*P_prev; my_mul+1:
    P = max(v, l) with a_flop; lane _V_LANE captures v for the tail.
    Shared tail: slice5 s = v - l; slice6 h = max(s, 0); slice7 bypass."""
    u = _base_uop()
    u.require_inp0 = 1
    u.repeat_count = 1
    u.trigger = (Trigger.SRC_TENSOR_DONE, Trigger.COUNT, Trigger.NONE)
    u.next_uop = (0, next_idx, 0)
    dp = u.datapath_config
    for k in range(my_mul):
        dp[k].pass_through_delay(_LANE_X, _LANE_W, _LANE_Z)
    dp[my_mul].enable_alu(AluOp.MULTIPLY, AluInp.PREV_DELAY_0 + _LANE_W,
                          AluInp.NEXT_ALU_OUT_A)
    dp[my_mul].pass_through_delay(_LANE_X, _LANE_Z)
    pmax = dp[my_mul + 1].enable_alu(AluOp.MAX, AluInp.PREV_ALU_OUT,
                                     AluInp.PREV_DELAY_0 + _LANE_X)
    pmax.alu_out_a_enable = ENABLE
    pmax.enable_delay_from_src(DelayInp.PREV_ALU_OUT, _V_LANE)  # v
    pmax.pass_through_delay(_LANE_X, _LANE_Z)
    for k in range(my_mul + 2, 5):
        dp[k].pass_through_delay(_LANE_X, _LANE_Z, _V_LANE)
    dp[5].enable_alu(AluOp.SUBTRACT, AluInp.PREV_DELAY_0 + _V_LANE,
                     AluInp.PREV_DELAY_0 + _LANE_X)
    dp[5].pass_through_delay(_LANE_Z)
    dp[6].enable_alu(AluOp.MAX, AluInp.PREV_ALU_OUT,
                     AluInp.PREV_DELAY_0 + _LANE_Z)
    _finish_bypass(u, 7)
    return u


def _lscan2_uops():
    seed = _seed_uop(next_idx=1)
    ua = _lscan2_steady(my_mul=1, other_flop_wr=4, next_idx=2)
    ub = _lscan2_steady(my_mul=3, other_flop_wr=2, next_idx=1)
    return [seed, ua, ub]


def _qrec2_uops():
    seed = _seed_uop(next_idx=1)
    ua = _qrec2_steady(my_mul=1, next_idx=2)
    ub = _qrec2_steady(my_mul=3, next_idx=1)
    return [seed, ua, ub]


def _ref_lscan2(in0, in1, c0, c1, c2):
    x = np.asarray(in0, np.float32).reshape(in0.shape[0], -1, 2)
    w = np.asarray(c0, np.float32).reshape(-1)
    out = np.empty_like(x)
    for s in range(2):
        acc = np.zeros_like(w)
        for t in range(x.shape[1]):
            acc = w * acc + x[:, t, s]
            out[:, t, s] = acc
    return out.reshape(in0.shape)


def _ref_qrec2(in0, in1, c0, c1, c2):
    el = np.asarray(in0, np.float32).reshape(in0.shape[0], -1, 2)
    w = np.asarray(c0, np.float32).reshape(-1)
    out = np.empty_like(el)
    for s in range(2):
        P_ = np.zeros_like(w)
        for t in range(el.shape[1]):
            v = w * P_
            P_ = np.maximum(el[:, t, s], v)
            out[:, t, s] = np.maximum(v - el[:, t, s], 0.0)
    return out.reshape(in0.shape)


class _HandOp:
    """Duck-typed DveOp with a hand-built uop program (bypasses lower())."""

    def __init__(self, name, spec, uops):
        self.name = name
        self.spec = spec
        self.subdim = False
        self._uops = uops
        self._cache = {}

    def compile(self, ver):
        assert ver == "v3", f"hand-built uops are v3-only, got {ver}"
        if ver not in self._cache:
            s = DveOpSpec(name=self.name,
                          opcode=dve_ops.get_dve_sub_opcode(self.name),
                          uops=self._uops, rd1_en=False)
            s.validate(ver)
            self._cache[ver] = s
        return self._cache[ver]


def _register():
    if "ANT_LSCAN2" in dve_ops._SUB_OPCODE_FOR_NAME:
        import sys
        mod = sys.modules.get("ant_irnn_ops_registered")
        return mod.LSCAN2, mod.QREC2
    spec1 = Spec(body=sp_relu(Src0 * C0), reference=_ref_lscan2)
    spec2 = Spec(body=sp_relu(Src0 * C0), reference=_ref_qrec2)
    ls = _HandOp("ANT_LSCAN2", spec1, _lscan2_uops())
    qr = _HandOp("ANT_QREC2", spec2, _qrec2_uops())
    base = max(dve_ops._SUB_OPCODE_FOR_NAME.values())
    dve_ops._SUB_OPCODE_FOR_NAME[ls.name] = base + 1
    dve_ops._SUB_OPCODE_FOR_NAME[qr.name] = base + 2
    assert max(dve_ops._SUB_OPCODE_FOR_NAME.values()) < 0x20
    dve_ops.OPS.append(ls)
    dve_ops.OPS.append(qr)
    dve_ops.CUSTOM_DVE_SPECS[ls.name] = ls.spec
    dve_ops.CUSTOM_DVE_SPECS[qr.name] = qr.spec
    import sys, types
    mod = types.ModuleType("ant_irnn_ops_registered")
    mod.LSCAN2, mod.QREC2 = ls, qr
    sys.modules["ant_irnn_ops_registered"] = mod
    return ls, qr


LSCAN2, QREC2 = _register()

# --- kernel ---


import numpy as np
from contextlib import ExitStack

import concourse.bass as bass
import concourse.tile as tile
from concourse import mybir
from concourse.bass_utils import run_bass_kernel_spmd


dt = mybir.dt
Alu = mybir.AluOpType
Act = mybir.ActivationFunctionType

B, T, I, H, L = 32, 2048, 256, 512, 4
NCORES = 8
BLOC = B // NCORES
P = 128
TCH = 512
M4 = H // P
KI = I // P


def build(include_bias=False):
    nc = bass.Bass("TRN2", target_bir_lowering=False, debug=False,
                   num_devices=NCORES)
    xT_d = nc.dram_tensor("xT", [BLOC, I, T], dt.float16, kind="ExternalInput").ap()
    w0_d = nc.dram_tensor("w0tn", [I, H], dt.float16, kind="ExternalInput").ap()
    ws_d = nc.dram_tensor("wstn", [L - 1, H, H], dt.float16, kind="ExternalInput").ap()
    wq_d = nc.dram_tensor("wq", [P, L * M4], dt.float32, kind="ExternalInput").ap()
    bias_d = nc.dram_tensor("biasn", [L, 1, H], dt.float16, kind="ExternalInput").ap()
    # interleaved batch-pair output: [pair, H, (t b)] -- host de-interleaves
    out_d = nc.dram_tensor("out", [BLOC // 2, H, 2 * T], dt.float16,
                           kind="ExternalOutput").ap()

    with tile.TileContext(nc) as tc, ExitStack() as ctx:
        wpool = ctx.enter_context(tc.tile_pool(name="weights", bufs=1))
        xpool = ctx.enter_context(tc.tile_pool(name="xin", bufs=1))
        spool = ctx.enter_context(tc.tile_pool(name="stage", bufs=1))
        psum = ctx.enter_context(tc.tile_pool(name="psum", bufs=2, space="PSUM"))

        # ---- persistent weights (DMA priority order: wq, w0, xin b0/b1,
        #      then xin b2/b3, then layer 1-3 weights) ----
        wq_dmas, crit_dmas, late_dmas = [], [], []
        wqall = wpool.tile([P, L * M4], dt.float32, tag="wqall")
        wq_dmas.append(nc.gpsimd.dma_start(out=wqall[:], in_=wq_d))
        wq = [[wqall[:, (l * M4 + m):(l * M4 + m) + 1] for m in range(M4)]
              for l in range(L)]
        wt = [[] for _ in range(L)]
        for k in range(KI):
            w = wpool.tile([P, H], dt.float16, tag=f"w0{k}")
            crit_dmas.append(nc.gpsimd.dma_start(
                out=w[:], in_=w0_d[k * P:(k + 1) * P, :]))
            wt[0].append(w)
        # split each input tile load into 4 chunks so the critical first
        # batch-pair spreads across all SWDGE queues (per-queue BW ~22GB/s)
        xin = [[] for _ in range(BLOC)]
        xin_dmas = [[] for _ in range(BLOC)]
        for b in range(BLOC):
            for k in range(KI):
                xt = xpool.tile([P, T], dt.float16, tag=f"x{b}{k}")
                xin[b].append(xt)
        # critical loads (b0/b1) split into 4 chunks each so the pipeline can
        # start on chunk 0; late loads (b2/b3) full-tile afterwards
        for c in range(4):
            cs = slice(c * TCH, (c + 1) * TCH)
            for b in (0, 1):
                for k in range(KI):
                    d = nc.gpsimd.dma_start(out=xin[b][k][:, cs],
                                            in_=xT_d[b, k * P:(k + 1) * P, cs])
                    crit_dmas.append(d)
                    xin_dmas[b].append((c, d))
        for b in (2, 3):
            for k in range(KI):
                d = nc.gpsimd.dma_start(out=xin[b][k][:],
                                        in_=xT_d[b, k * P:(k + 1) * P, :])
                late_dmas.append(d)
                xin_dmas[b].append((-1, d))
        ws_dmas = []
        for l in range(1, L):
            for k in range(M4):
                w = wpool.tile([P, H], dt.float16, tag=f"w{l}{k}")
                d = nc.gpsimd.dma_start(
                    out=w[:], in_=ws_d[l - 1, k * P:(k + 1) * P, :])
                ws_dmas.append(d)
                wt[l].append(w)
        bias = None
        if include_bias:
            bias = []
            for l in range(L):
                bt = wpool.tile([1, H], dt.float16, tag=f"b{l}")
                ws_dmas.append(nc.gpsimd.dma_start(out=bt[:],
                                                   in_=bias_d[l, :, :]))
                bias.append(bt)
            ones = wpool.tile([1, TCH], dt.float16, tag="ones")
            nc.gpsimd.memset(ones[:], 1.0)

        # ---- absorber machinery (per-engine pinned chains) ----
        scr_v = wpool.tile([P, 160], dt.float32, tag="scr_v")
        scr_a = wpool.tile([P, 160], dt.float32, tag="scr_a")
        state = {"V": [None, 0], "A": [None, 0], "PE": [None]}

        def absorb(eng, dep=None):
            if eng == "V":
                prev, k = state[eng]
                i = nc.vector.tensor_copy(scr_v[:, k:k + 1], wq[0][0][:])
            elif eng == "A":
                prev, k = state[eng]
                i = nc.scalar.activation(scr_a[:, k:k + 1], wq[0][0][:],
                                         Act.Copy)
            else:
                prev = state[eng][0]
                i = nc.tensor.ldweights(weights=wt[0][0][:, 0:P])
            if prev is not None:
                bass._add_dep_helper(i.ins, prev.ins, sync=False, reason="chain")
            if dep is not None:
                bass._add_dep_helper(i.ins, dep.ins, sync=True, reason="absorb")
            if eng == "PE":
                state[eng] = [i]
            else:
                state[eng] = [i, (state[eng][1] + 1) % 160]
            return i

        def pin(real, eng):
            prev = state[eng][0]
            if prev is not None:
                bass._add_dep_helper(real.ins, prev.ins, sync=False, reason="pin")
            state[eng][0] = real
            return real

        # warm-up: junk compute while input DMAs land (p-state ramp)
        jw = wpool.tile([P, P], dt.float16, tag="jw")
        jx = wpool.tile([P, 2 * T], dt.float16, tag="jx")
        jo = wpool.tile([P, 2 * T], dt.float16, tag="jo")
        jq = wpool.tile([P, 1], dt.float32, tag="jq")
        nc.vector.memset(jw[:, 0:1], 0.125)
        nc.vector.memset(jx[:, 0:1], 0.125)
        nc.vector.memset(jq[:], 0.125)
        for s in range(2):
            jp = psum.tile([P, T], dt.float32, tag="xp")
            for r in range(12):
                pin(nc.tensor.matmul(jp[:, 0:TCH], lhsT=jw[:],
                                     rhs=jx[:, 0:TCH],
                                     start=True, stop=True), "PE")
        for r in range(3):
            pin(nc.vector._custom_dve(LSCAN2, out=jo[:], in0=jx[:],
                                      s0=jq[:]), "V")

        # engine init: V/A see only the (tiny, first-issued) wq DMAs; PE
        # preloads just the layer-0 weights. Everything else is absorbed
        # just-in-time so the pipeline starts as soon as b0/b1 inputs land.
        for eng in ("V", "A"):
            absorb(eng)
            for d in wq_dmas:
                absorb(eng, d)
        for k in range(KI):
            pin(nc.tensor.ldweights(weights=wt[0][k][:, 0:P]), "PE")
        if include_bias:
            for l in range(L):
                pin(nc.tensor.ldweights(weights=bias[l][:, 0:P]), "PE")
            pin(nc.tensor.ldweights(weights=ones[:, 0:P]), "PE")

        # ---- main loop ----
        # stage rings
        NXSB, NHP = 3, 13
        hp_ring = []   # list of (key, tile); key=(bp, l, m)
        hp_by_key = {}
        hp_readers = {}   # key -> last matmul reading it
        xsb_readers = [None] * NXSB  # ring idx -> LSCAN2 that read it
        psum_readers = [None, None]  # psum slot -> ACT copy that read it
        xsb_i = 0
        psum_i = 0
        hp_i = 0

        last_qrec = {}
        for l in range(L):
            kprev = KI if l == 0 else M4
            if l > 0:
                # absorb the layer's weight DMAs into junk ldweights now --
                # the loads finished long ago, so no stall
                for k in range(M4):
                    pin(nc.tensor.ldweights(weights=wt[l][k][:, 0:P]), "PE")
            for bp in range(BLOC // 2):
                if l > 0:
                    # PE absorbs the DVE tick of the last h producer for this
                    # (bp, l-1) so real matmuls carry no DVE wait
                    absorb("PE", last_qrec[bp])
                else:
                    for b in (2 * bp, 2 * bp + 1):
                        for c, d in xin_dmas[b]:
                            if c < 0:
                                absorb("PE", d)
                for m in range(M4):
                    xsb = spool.tile([P, 2 * T], dt.float16, tag="xsb",
                                     bufs=NXSB)
                    xsb_slot = xsb_i % NXSB
                    xsb_i += 1
                    copies = []
                    for half in range(2):
                        b = 2 * bp + half
                        xp = psum.tile([P, T], dt.float32, tag="xp")
                        slot = psum_i % 2
                        psum_i += 1
                        # PE claimer: absorb the ACT copy that last read this
                        # PSUM slot (WAR) before overwriting it
                        old_rd = psum_readers[slot]
                        if old_rd is not None:
                            absorb("PE", old_rd)
                        last_mm = None
                        for n in range(T // TCH):
                            ns = slice(n * TCH, (n + 1) * TCH)
                            if l == 0:
                                for c, d in xin_dmas[b]:
                                    if c == n:
                                        absorb("PE", d)
                            for k in range(kprev):
                                if l == 0:
                                    rhs = xin[b][k][:, ns]
                                else:
                                    hsrc = hp_by_key[(bp, l - 1, k)]
                                    rhs = hsrc[:, 2 * n * TCH + half:
                                               2 * (n + 1) * TCH:2]
                                last_mm = nc.tensor.matmul(
                                    xp[:, ns], lhsT=wt[l][k][:, m * P:(m + 1) * P],
                                    rhs=rhs, start=(k == 0),
                                    stop=(k == kprev - 1 and not include_bias))
                                pin(last_mm, "PE")
                                if l > 0:
                                    hp_readers[(bp, l - 1, k)] = last_mm
                            if include_bias:
                                last_mm = pin(nc.tensor.matmul(
                                    xp[:, ns], lhsT=bias[l][:, m * P:(m + 1) * P],
                                    rhs=ones[:, :], start=False, stop=True), "PE")
                        # ACT: de-stride copy PSUM fp32 -> xsb half (fp16)
                        # absorb the xsb-slot WAR (old LSCAN2 reader, DVE)
                        old_x = xsb_readers[xsb_slot]
                        if half == 0 and old_x is not None:
                            absorb("A", old_x)
                        absorb("A", last_mm)
                        cp = pin(nc.scalar.activation(
                            xsb[:].rearrange("p (t s) -> p s t", s=2)[
                                :, half:half + 1, :],
                            xp[:].rearrange("p (o t) -> p o t", o=1),
                            Act.Copy), "A")
                        psum_readers[slot] = cp
                        copies.append(cp)
                    # DVE: LSCAN2 + QREC2 (custom ops must carry NO foreign
                    # sync waits -> absorb the ACT copies first)
                    absorb("V", copies[-1])
                    lp = spool.tile([P, 2 * T], dt.float16, tag="lp", bufs=2)
                    ls = pin(nc.vector._custom_dve(
                        LSCAN2, out=lp[:], in0=xsb[:],
                        s0=wq[l][m][:]), "V")
                    xsb_readers[xsb_slot] = ls
                    hpt = spool.tile([P, 2 * T], dt.float16, tag="hp",
                                     bufs=NHP)
                    # hp ring WAR: absorb the last matmul that read the
                    # evicted hp tile
                    if len(hp_ring) >= NHP:
                        old_key = hp_ring[hp_i % NHP]
                        rd = hp_readers.pop(old_key, None)
                        if rd is not None:
                            absorb("V", rd)
                    if len(hp_ring) < NHP:
                        hp_ring.append((bp, l, m))
                    else:
                        hp_ring[hp_i % NHP] = (bp, l, m)
                    hp_i += 1
                    hp_by_key[(bp, l, m)] = hpt
                    qr = pin(nc.vector._custom_dve(
                        QREC2, out=hpt[:], in0=lp[:],
                        s0=wq[l][m][:]), "V")
                    last_qrec[bp] = qr
                    if l == L - 1:
                        # store the interleaved pair as-is; host de-interleaves
                        nc.sync.dma_start(
                            out=out_d[bp, m * P:(m + 1) * P, :], in_=hpt[:])

        # ---- tail pre-drains ----
        tail_deps = [i for i in nc.inst_map.values()
                     if type(i).__name__ == "InstDMACopy"]
        snap = list(nc.inst_map.values())
        for eng in ("DVE", "Activation", "PE"):
            last_e = [i for i in snap
                      if str(getattr(i, "engine", "")).endswith(eng)]
            if last_e:
                tail_deps.append(last_e[-1])
        for depi in tail_deps:
            dr = nc.sync.drain(fusable=False)
            bass._add_dep_helper(dr.ins, depi, sync=True,
                                 reason="tail pre-drain absorber")
    assert mybir.codegen_inst_isa_subclasses(nc)
    _assert_wait_budget(nc)
    return nc


_MULTI_WAIT_OK = {"InstDrain",
                  "InstEventSemaphore", "InstUnconditionalBranch",
                  "InstRegisterMove", "InstISA", "InstTensorLoad",
                  "InstTensorSave"}


def _assert_wait_budget(nc):
    bad = []
    for name, inst in nc.inst_map.items():
        ty = type(inst).__name__
        w = inst.sync_info.on_wait if inst.sync_info else []
        if ty == "InstCustomDveAnt":
            # custom-dve = raw InstISA at walrus: no foreign semaphore waits
            fw = [x for x in w if not x.ant_name.startswith("DVE")]
            if fw:
                bad.append((name, ty, [f"{x.ant_name}>={x.wait_value}"
                                       for x in fw]))
            continue
        if ty in _MULTI_WAIT_OK:
            continue
        if len(w) > 1:
            bad.append((name, ty,
                        [f"{x.ant_name}>={x.wait_value}" for x in w]))
    if bad:
        raise RuntimeError(
            f"{len(bad)} instructions exceed the sync-wait budget, "
            f"first few: {bad[:6]}")


def _prep_core_inputs(Input, W0, Ws, bs, whs, core):
    bsl = slice(core * BLOC, (core + 1) * BLOC)
    return {
        "xT": np.ascontiguousarray(
            Input[bsl].transpose(0, 2, 1)).astype(np.float16),
        "w0tn": np.ascontiguousarray(-W0.T).astype(np.float16),
        "wstn": np.ascontiguousarray(-Ws.transpose(0, 2, 1)).astype(np.float16),
        "wq": np.ascontiguousarray(
            whs.astype(np.float32).reshape(L, M4, P).transpose(2, 0, 1)
            .reshape(P, L * M4)),
        "biasn": np.ascontiguousarray(-bs[:, None, :]).astype(np.float16),
    }


def kernel(Input, W0, Ws, bs, whs):
    include_bias = bool(np.any(bs != 0))
    nc = build(include_bias=include_bias)
    in_maps = [_prep_core_inputs(Input, W0, Ws, bs, whs, r)
               for r in range(NCORES)]
    res = run_bass_kernel_spmd(nc, in_maps, core_ids=list(range(NCORES)))
    parts = []
    for r in range(NCORES):
        o = res.results[r]["out"]  # [BLOC//2, H, 2T] interleaved pairs
        o = o.reshape(BLOC // 2, H, T, 2).transpose(0, 3, 1, 2)
        parts.append(o.reshape(BLOC, H, T))
    full = np.concatenate(parts, axis=0)  # [B, H, T]
    return np.ascontiguousarray(full.transpose(0, 2, 1)).astype(np.float32)

